# revision 9
# baseline (speedup 1.0000x reference)
"""MPNEncoder (chemprop D-MPNN) Trainium2 kernel, 8-core SPMD.

Sharding: bonds 8x50000, atoms 8x25000 (data-parallel). Each core holds a
replicated message table in DRAM, refreshed by AllGather after each
message-passing step. Graph gathers run as SWDGE indirect DMAs with
host-precomputed int32 index tiles. Readout pooling is a one-hot matmul
into per-call 128-molecule windows; host adds the per-core partial sums.
"""
import os
import sys
import time

sys.path.insert(0, "/opt/trn_rl_repo")
import numpy as np
import ml_dtypes

import concourse.bass as bass
import concourse.mybir as mybir
import concourse.tile as tile
from concourse.bass_utils import run_bass_kernel_spmd
from concourse.masks import make_identity

# ---- problem constants (hardcoded per spec) ----
NB, NA, NM = 400000, 200000, 10000
HID, BFD, AFD = 300, 147, 133
DEPTH = 3
NCORES = 8
SB, SA = NB // NCORES, NA // NCORES  # 50000, 25000
TB = 8                     # bonds per partition per update call -> 1024/call
NCALL_U = 50               # update calls; SBP = 50*1024
SBP = NCALL_U * 128 * TB   # 51200
TA = 8                     # atoms per partition per readout call
NCALL_R = 25               # readout calls; SAP = 25*1024
SAP = NCALL_R * 128 * TA   # 25600

BF16 = True
NPDT = ml_dtypes.bfloat16 if BF16 else np.float32
MDT = mybir.dt.bfloat16 if BF16 else mybir.dt.float32
F32 = mybir.dt.float32
I32 = mybir.dt.int32

_CACHED_NC = None
LAST_DEBUG = None


def _split_excess_waits(nc, max_waits=1):
    """walrus caps sync waits per instruction; offload excess onto nops."""
    for bbname, bbw in nc.bb_map.items():
        bb = bbw.bb
        il = bb.instructions
        k = 0
        while k < len(il):
            inst = il[k]
            si = inst.sync_info
            if si is None or len(si.on_wait) <= max_waits:
                k += 1
                continue
            ow = list(si.on_wait)
            keep, rest = ow[-max_waits:], ow[:-max_waits]
            si.on_wait = keep
            eng = nc.engines[inst.engine]
            pos = k
            while rest:
                chunk, rest = rest[:max_waits], rest[max_waits:]
                ni = eng.nop(nofuse=True)
                ni_inst = getattr(ni, "ins", ni)
                cur = nc.cur_bb.bb.instructions
                assert cur[-1] is ni_inst, "nop landed elsewhere"
                cur.pop()
                ni_inst.sync_info = mybir.SyncInfo(on_wait=chunk, on_update=[])
                il.insert(pos, ni_inst)
                pos += 1
                k += 1
            k += 1


def _build():
    nc = bass.Bass(num_devices=NCORES)
    fbT = nc.declare_dram_parameter("fbT", [BFD, SBP], MDT, isOutput=False)
    faT = nc.declare_dram_parameter("faT", [AFD, SAP], MDT, isOutput=False)
    g1 = nc.declare_dram_parameter("g1", [NCALL_R, 128, TA * 6], I32, isOutput=False)
    g2p = nc.declare_dram_parameter("g2p", [NCALL_U, 128, TB * 6], I32, isOutput=False)
    g3 = nc.declare_dram_parameter("g3", [NCALL_U, 128, TB], I32, isOutput=False)
    oh = nc.declare_dram_parameter("oh", [NCALL_R * TA, 128, 128], MDT, isOutput=False)
    Wi = nc.declare_dram_parameter("Wi", [BFD, HID], MDT, isOutput=False)
    Wh = nc.declare_dram_parameter("Wh", [HID, HID], MDT, isOutput=False)
    Woa = nc.declare_dram_parameter("Woa", [AFD, HID], MDT, isOutput=False)
    Wom = nc.declare_dram_parameter("Wom", [HID, HID], MDT, isOutput=False)
    bob = nc.declare_dram_parameter("bob", [128, HID], F32, isOutput=False)
    pool_out = nc.declare_dram_parameter(
        "pool_out", [NCALL_R, 128, HID], F32, isOutput=True
    )
    mdbg = nc.declare_dram_parameter("mdbg", [DEPTH, 1024, HID], MDT, isOutput=True)

    rg = [list(range(NCORES))]
    relu = mybir.ActivationFunctionType.Relu
    sub = mybir.AluOpType.subtract

    with tile.TileContext(nc) as tc:
        with (
            tc.tile_pool(name="const", bufs=1) as cp,
            tc.tile_pool(name="dram", bufs=1, space="DRAM") as dram,
        ):
            ident = cp.tile([128, 128], MDT)
            make_identity(nc, ident[:])
            wi0 = cp.tile([128, HID], MDT)
            wi1 = cp.tile([BFD - 128, HID], MDT)
            nc.sync.dma_start(out=wi0[:], in_=Wi[0:128, :])
            nc.sync.dma_start(out=wi1[:], in_=Wi[128:BFD, :])
            wh0 = cp.tile([128, HID], MDT)
            wh1 = cp.tile([128, HID], MDT)
            wh2 = cp.tile([HID - 256, HID], MDT)
            nc.sync.dma_start(out=wh0[:], in_=Wh[0:128, :])
            nc.sync.dma_start(out=wh1[:], in_=Wh[128:256, :])
            nc.sync.dma_start(out=wh2[:], in_=Wh[256:HID, :])
            wom0 = cp.tile([128, HID], MDT)
            wom1 = cp.tile([128, HID], MDT)
            wom2 = cp.tile([HID - 256, HID], MDT)
            nc.sync.dma_start(out=wom0[:], in_=Wom[0:128, :])
            nc.sync.dma_start(out=wom1[:], in_=Wom[128:256, :])
            nc.sync.dma_start(out=wom2[:], in_=Wom[256:HID, :])
            woa0 = cp.tile([128, HID], MDT)
            woa1 = cp.tile([AFD - 128, HID], MDT)
            nc.sync.dma_start(out=woa0[:], in_=Woa[0:128, :])
            nc.sync.dma_start(out=woa1[:], in_=Woa[128:AFD, :])
            bob_t = cp.tile([128, HID], F32)
            nc.sync.dma_start(out=bob_t[:], in_=bob[:])

            M_shard = dram.tile([SBP, HID], MDT)
            M_full = dram.tile([NB, HID], MDT)
            inp = dram.tile([SBP, HID], F32)

            # ---- phase 0: inp = f_bonds @ W_i ; message = relu(inp) ----
            with (
                tc.tile_pool(name="p0", bufs=3) as p0,
                tc.tile_pool(name="ps0", bufs=2, space="PSUM") as ps0,
            ):
                for k in range(SBP // 128):
                    s = slice(k * 128, (k + 1) * 128)
                    fb0 = p0.tile([128, 128], MDT, tag="fb0")
                    fb1 = p0.tile([BFD - 128, 128], MDT, tag="fb1")
                    nc.sync.dma_start(out=fb0[:], in_=fbT[0:128, s])
                    nc.sync.dma_start(out=fb1[:], in_=fbT[128:BFD, s])
                    pm = ps0.tile([128, HID], F32, tag="pm")
                    nc.tensor.matmul(
                        out=pm[:], lhsT=fb0[:], rhs=wi0[:], start=True, stop=False
                    )
                    nc.tensor.matmul(
                        out=pm[:], lhsT=fb1[:], rhs=wi1[:], start=False, stop=True
                    )
                    it_ = p0.tile([128, HID], F32, tag="it")
                    nc.vector.tensor_copy(out=it_[:], in_=pm[:])
                    ms = p0.tile([128, HID], MDT, tag="ms")
                    nc.scalar.activation(ms[:], pm[:], relu)
                    nc.sync.dma_start(out=inp[s, :], in_=it_[:])
                    nc.sync.dma_start(out=M_shard[s, :], in_=ms[:])

            nc.gpsimd.collective_compute(
                "AllGather",
                mybir.AluOpType.bypass,
                replica_groups=rg,
                ins=[M_shard[0:SB, :]],
                outs=[M_full[:]],
            )

            nc.sync.dma_start(out=mdbg[0], in_=M_full[0:1024, :])

            # ---- message passing iterations ----
            for _it in range(DEPTH - 1):
                with (
                    tc.tile_pool(name="up", bufs=2) as up,
                    tc.tile_pool(name="ups", bufs=2, space="PSUM") as ups,
                ):
                    for j in range(NCALL_U):
                        i2 = up.tile([128, TB * 6], I32, tag="i2")
                        nc.sync.dma_start(out=i2[:], in_=g2p[j])
                        i3 = up.tile([128, TB], I32, tag="i3")
                        nc.sync.dma_start(out=i3[:], in_=g3[j])
                        g2t = up.tile([128, TB, 6, HID], MDT, tag="g2t")
                        nc.gpsimd.indirect_dma_start(
                            out=g2t[:].rearrange("p t k h -> p (t k h)"),
                            out_offset=None,
                            in_=M_full[:],
                            in_offset=bass.IndirectOffsetOnAxis(ap=i2[:], axis=0),
                        )
                        g3t = up.tile([128, TB, HID], MDT, tag="g3t")
                        nc.gpsimd.indirect_dma_start(
                            out=g3t[:].rearrange("p t h -> p (t h)"),
                            out_offset=None,
                            in_=M_full[:],
                            in_offset=bass.IndirectOffsetOnAxis(ap=i3[:], axis=0),
                        )
                        dl = up.tile([128, TB, HID], MDT, tag="dl")
                        nc.vector.tensor_add(
                            out=dl[:], in0=g2t[:, :, 0, :], in1=g2t[:, :, 1, :]
                        )
                        for nb in range(2, 6):
                            nc.vector.tensor_add(
                                out=dl[:], in0=dl[:], in1=g2t[:, :, nb, :]
                            )
                        nc.vector.tensor_tensor(
                            out=dl[:], in0=dl[:], in1=g3t[:], op=sub
                        )
                        ip = up.tile([128, TB, HID], F32, tag="ip")
                        bs = slice(j * 128 * TB, (j + 1) * 128 * TB)
                        nc.sync.dma_start(
                            out=ip[:],
                            in_=inp[bs, :].rearrange("(p t) h -> p t h", t=TB),
                        )
                        mo = up.tile([128, TB, HID], MDT, tag="mo")
                        for t in range(TB):
                            pT = ups.tile([128, 3, 128], MDT, tag="pT")
                            nc.tensor.transpose(pT[:, 0, :], dl[:, t, 0:128], ident[:])
                            nc.tensor.transpose(
                                pT[:, 1, :], dl[:, t, 128:256], ident[:]
                            )
                            nc.tensor.transpose(
                                pT[0 : HID - 256, 2, :], dl[:, t, 256:HID], ident[:]
                            )
                            dT = up.tile([128, 3, 128], MDT, tag="dT")
                            nc.vector.tensor_copy(
                                out=dT[:, 0:2, :], in_=pT[:, 0:2, :]
                            )
                            nc.vector.tensor_copy(
                                out=dT[0 : HID - 256, 2, :], in_=pT[0 : HID - 256, 2, :]
                            )
                            po = ups.tile([128, HID], F32, tag="po")
                            nc.tensor.matmul(
                                out=po[:], lhsT=dT[:, 0, :], rhs=wh0[:],
                                start=True, stop=False,
                            )
                            nc.tensor.matmul(
                                out=po[:], lhsT=dT[:, 1, :], rhs=wh1[:],
                                start=False, stop=False,
                            )
                            nc.tensor.matmul(
                                out=po[:],
                                lhsT=dT[0 : HID - 256, 2, :],
                                rhs=wh2[:],
                                start=False, stop=True,
                            )
                            nc.vector.tensor_add(
                                out=po[:], in0=po[:], in1=ip[:, t, :]
                            )
                            nc.scalar.activation(mo[:, t, :], po[:], relu)
                        nc.sync.dma_start(
                            out=M_shard[bs, :].rearrange("(p t) h -> p t h", t=TB),
                            in_=mo[:],
                        )
                nc.gpsimd.collective_compute(
                    "AllGather",
                    mybir.AluOpType.bypass,
                    replica_groups=rg,
                    ins=[M_shard[0:SB, :]],
                    outs=[M_full[:]],
                )
                nc.sync.dma_start(out=mdbg[1 + _it], in_=M_full[0:1024, :])

            # ---- readout ----
            with (
                tc.tile_pool(name="ro", bufs=2) as ro,
                tc.tile_pool(name="ros", bufs=2, space="PSUM") as ros,
                tc.tile_pool(name="rop", bufs=2, space="PSUM") as rop,
            ):
                for j in range(NCALL_R):
                    i1 = ro.tile([128, TA * 6], I32, tag="i1")
                    nc.sync.dma_start(out=i1[:], in_=g1[j])
                    g1t = ro.tile([128, TA, 6, HID], MDT, tag="g1t")
                    nc.gpsimd.indirect_dma_start(
                        out=g1t[:].rearrange("p t k h -> p (t k h)"),
                        out_offset=None,
                        in_=M_full[:],
                        in_offset=bass.IndirectOffsetOnAxis(ap=i1[:], axis=0),
                    )
                    am = ro.tile([128, TA, HID], MDT, tag="am")
                    nc.vector.tensor_add(
                        out=am[:], in0=g1t[:, :, 0, :], in1=g1t[:, :, 1, :]
                    )
                    for nb in range(2, 6):
                        nc.vector.tensor_add(out=am[:], in0=am[:], in1=g1t[:, :, nb, :])
                    php = rop.tile([128, HID], F32, tag="php")
                    for t in range(TA):
                        pT = ros.tile([128, 3, 128], MDT, tag="pTr")
                        nc.tensor.transpose(pT[:, 0, :], am[:, t, 0:128], ident[:])
                        nc.tensor.transpose(pT[:, 1, :], am[:, t, 128:256], ident[:])
                        nc.tensor.transpose(
                            pT[0 : HID - 256, 2, :], am[:, t, 256:HID], ident[:]
                        )
                        aT = ro.tile([128, 3, 128], MDT, tag="aT")
                        nc.vector.tensor_copy(out=aT[:, 0:2, :], in_=pT[:, 0:2, :])
                        nc.vector.tensor_copy(
                            out=aT[0 : HID - 256, 2, :], in_=pT[0 : HID - 256, 2, :]
                        )
                        cs = slice(j * 128 * TA + t * 128, j * 128 * TA + (t + 1) * 128)
                        fa0 = ro.tile([128, 128], MDT, tag="fa0")
                        fa1 = ro.tile([AFD - 128, 128], MDT, tag="fa1")
                        nc.sync.dma_start(out=fa0[:], in_=faT[0:128, cs])
                        nc.sync.dma_start(out=fa1[:], in_=faT[128:AFD, cs])
                        pa = ros.tile([128, HID], F32, tag="pa")
                        nc.tensor.matmul(
                            out=pa[:], lhsT=aT[:, 0, :], rhs=wom0[:],
                            start=True, stop=False,
                        )
                        nc.tensor.matmul(
                            out=pa[:], lhsT=aT[:, 1, :], rhs=wom1[:],
                            start=False, stop=False,
                        )
                        nc.tensor.matmul(
                            out=pa[:], lhsT=aT[0 : HID - 256, 2, :], rhs=wom2[:],
                            start=False, stop=False,
                        )
                        nc.tensor.matmul(
                            out=pa[:], lhsT=fa0[:], rhs=woa0[:],
                            start=False, stop=False,
                        )
                        nc.tensor.matmul(
                            out=pa[:], lhsT=fa1[:], rhs=woa1[:],
                            start=False, stop=True,
                        )
                        nc.vector.tensor_add(out=pa[:], in0=pa[:], in1=bob_t[:])
                        ah = ro.tile([128, HID], MDT, tag="ah")
                        nc.scalar.activation(ah[:], pa[:], relu)
                        oht = ro.tile([128, 128], MDT, tag="oht")
                        nc.sync.dma_start(out=oht[:], in_=oh[j * TA + t])
                        nc.tensor.matmul(
                            out=php[:],
                            lhsT=oht[:],
                            rhs=ah[:],
                            start=(t == 0),
                            stop=(t == TA - 1),
                        )
                    ps = ro.tile([128, HID], F32, tag="psout")
                    nc.vector.tensor_copy(out=ps[:], in_=php[:])
                    nc.sync.dma_start(out=pool_out[j], in_=ps[:])
    return nc


def _prep_core(c, f_atoms, f_bonds, a2b, b2a, b2revb, atom2mol):
    """Host-side index/tensor prep for one core. Returns (in_map, mol0s)."""
    i32 = np.int32
    bond0 = c * SB
    bl = np.arange(SBP)
    gbc = np.where(bl < SB, bond0 + bl, bond0).astype(i32)
    src_atom = b2a[gbc]
    g2p = a2b[src_atom].reshape(NCALL_U, 128, TB * 6).astype(i32)
    g3 = b2revb[gbc].reshape(NCALL_U, 128, TB).astype(i32)
    fb = np.zeros((SBP, BFD), NPDT)
    fb[:SB] = f_bonds[bond0 : bond0 + SB].astype(NPDT)
    fbT = np.ascontiguousarray(fb.T)

    a0 = c * SA
    al = np.arange(SAP)
    avalid = al < SA
    gac = np.where(avalid, a0 + al, a0).astype(i32)
    g1 = a2b[gac].reshape(NCALL_R, 128, TA * 6).astype(i32)
    fa = np.zeros((SAP, AFD), NPDT)
    fa[:SA] = f_atoms[a0 : a0 + SA].astype(NPDT)
    # column order: j*1024 + t*128 + p  for atom id j*1024 + p*TA + t
    faT = np.ascontiguousarray(
        fa.reshape(NCALL_R, 128, TA, AFD).transpose(3, 0, 2, 1).reshape(AFD, SAP)
    )

    mols = atom2mol[gac].astype(np.int64)
    mv = np.where(avalid, mols, np.iinfo(np.int64).max).reshape(NCALL_R, 128, TA)
    mol0 = mv.min(axis=(1, 2))
    slot = (mols.reshape(NCALL_R, 128, TA) - mol0[:, None, None]).astype(np.int64)
    okm = avalid.reshape(NCALL_R, 128, TA)
    assert slot[okm].max() < 128, "molecule window overflow"
    oh = np.zeros((NCALL_R, TA, 128, 128), NPDT)
    jj, pp, tt = np.nonzero(okm)
    oh[jj, tt, pp, slot[jj, pp, tt]] = 1
    return (
        {
            "fbT": fbT,
            "faT": faT,
            "g1": g1,
            "g2p": g2p,
            "g3": g3,
            "oh": oh.reshape(NCALL_R * TA, 128, 128),
        },
        mol0,
    )


def kernel(f_atoms, f_bonds, W_i, W_h, W_o, b_o, a2b, b2a, b2revb, atom2mol, n_mols):
    global _CACHED_NC
    f_atoms = np.asarray(f_atoms, np.float32)
    f_bonds = np.asarray(f_bonds, np.float32)
    W_i = np.asarray(W_i, np.float32)
    W_h = np.asarray(W_h, np.float32)
    W_o = np.asarray(W_o, np.float32)
    b_o = np.asarray(b_o, np.float32)
    a2b = np.asarray(a2b, np.int32)
    b2a = np.asarray(b2a, np.int32)
    b2revb = np.asarray(b2revb, np.int32)
    atom2mol = np.asarray(atom2mol, np.int32)
    n_mols = int(n_mols)

    shared = {
        "Wi": W_i.astype(NPDT),
        "Wh": W_h.astype(NPDT),
        "Woa": W_o[:AFD].astype(NPDT),
        "Wom": W_o[AFD:].astype(NPDT),
        "bob": np.tile(b_o[None, :], (128, 1)).astype(np.float32),
    }
    in_maps = []
    mol0s = []
    for c in range(NCORES):
        m, mol0 = _prep_core(c, f_atoms, f_bonds, a2b, b2a, b2revb, atom2mol)
        m.update(shared)
        in_maps.append(m)
        mol0s.append(mol0)

    if _CACHED_NC is None:
        _CACHED_NC = _build()
        _split_excess_waits(_CACHED_NC)
    trace = os.environ.get("MPN_TRACE", "0") == "1"
    _t0 = time.time()
    res = run_bass_kernel_spmd(
        _CACHED_NC, in_maps, list(range(NCORES)), trace=trace
    )
    _wall_ns = int((time.time() - _t0) * 1e9)
    _hw = getattr(res, "exec_time_ns", None)
    print(f"HW exec time: {_hw if _hw is not None else _wall_ns} ns")

    global LAST_DEBUG
    LAST_DEBUG = [np.asarray(res.results[c].get("mdbg")) for c in range(NCORES)]
    sums = np.zeros((n_mols, HID), np.float32)
    for c in range(NCORES):
        po = np.asarray(res.results[c]["pool_out"], np.float32)
        for j in range(NCALL_R):
            m0 = int(mol0s[c][j])
            if m0 >= n_mols:
                continue
            n = min(128, n_mols - m0)
            sums[m0 : m0 + n] += po[j, :n]
    counts = np.bincount(atom2mol, minlength=n_mols).astype(np.float32)
    out = np.where(
        counts[:, None] > 0, sums / np.maximum(counts, 1.0)[:, None], 0.0
    ).astype(np.float32)
    return out
